# revision 11
# baseline (speedup 1.0000x reference)
"""Trainium2 Bass kernel for nn_Dconv_localshuffle.

Computes: per-channel spatial gather (block-local permutation within 4x4
blocks) followed by a 3x3 conv (stride 1, pad 1, no bias).

Sharding (8 NeuronCores, no cross-core comms):
  grid = (batch-half bh in {0,1}) x (row-quarter rq in {0..3})
  core k = bh*4 + rq handles images [bh*16, bh*16+16) and output rows
  [32*rq, 32*rq+32), all 64 channels. Halo rows are loaded redundantly
  (each core receives pre-sliced input), so cores are fully independent.

Per-core pipeline:
  DMA-in f32 tiles (partition = (ch-group, image)) ->
  GpSimd ap_gather (per-channel indices, shared across the 16 images on
  each Q7 core's partitions) -> DVE cast f32->bf16 ->
  SBUF reshuffle DMA into per-image zero-padded conv layout, duplicated
  on partitions 64-127 shifted one row down ->
  3x3 conv as 9 taps in 5 PE slots (3 fused K=128 dy-pairs + row-tiled
  pair + single), two images at a time via column tiling ->
  PSUM -> ACT evac -> DMA-out f32.
"""

import numpy as np

import concourse.bass as bass
import concourse.bacc as bacc
import concourse.mybir as mybir
from concourse import tile
from concourse.bass_utils import run_bass_kernel_spmd

# Problem constants (hardcoded per contract).
B, C, H, W = 32, 64, 128, 128
OUT = 64
N_CORES = 8
BH_IMGS = 16          # images per core
RQ_ROWS = 32          # output rows per core
SRC_ROWS = 40         # source rows loaded: [32*rq-4, 32*rq+36)
G_ROWS = 34           # gathered rows: [32*rq-1, 32*rq+33)
NE = SRC_ROWS * W     # 5120 gather source elems per partition
NIDX = G_ROWS * W     # 4352 gather indices per Q7 core
NIDX_W = NIDX // 16   # 272 wrapped idx columns
XPW = W + 2           # 130 padded row width
XPN = G_ROWS * XPW    # 4420 x_pad elems per partition

F32 = mybir.dt.float32
BF16 = mybir.dt.bfloat16
I16 = mybir.dt.int16

_CACHE = {}


def _build_nc():
    """Build + compile the (identical for all cores) Bass program."""
    nc = bacc.Bacc("TRN2", target_bir_lowering=False, debug=False,
                   num_devices=N_CORES)
    xs_d = nc.dram_tensor("xs", [8, 128, NE], F32, kind="ExternalInput")
    idx_d = nc.dram_tensor("idx16", [8, 128, NIDX_W], I16, kind="ExternalInput")
    wts_d = nc.dram_tensor("wts", [128, 6, OUT], BF16, kind="ExternalInput")
    out_d = nc.dram_tensor("out", [BH_IMGS, OUT, RQ_ROWS, W], F32,
                           kind="ExternalOutput")

    with tile.TileContext(nc) as tc:
        with (
            tc.tile_pool(name="const", bufs=1) as const_pool,
            tc.tile_pool(name="xs", bufs=2) as xs_pool,
            tc.tile_pool(name="gath", bufs=1) as gath_pool,
            tc.tile_pool(name="castall", bufs=1) as cast_pool,
            tc.tile_pool(name="xpad", bufs=3) as xpad_pool,
            tc.tile_pool(name="outsb", bufs=2) as out_pool,
            tc.tile_pool(name="psum", bufs=2, space=bass.MemorySpace.PSUM) as psum_pool,
        ):
            idx_sb = const_pool.tile([128, 8, NIDX_W], I16)
            for i in range(8):
                nc.sync.dma_start(out=idx_sb[:, i, :], in_=idx_d[i])
            wt_sb = const_pool.tile([128, 6, OUT], BF16)
            nc.sync.dma_start(out=wt_sb[:], in_=wts_d[:])

            # -------- gather phase: 8 channel-groups --------
            cast_all = cast_pool.tile([128, 8, NIDX], BF16)
            for i in range(8):
                xs_t = xs_pool.tile([128, NE], F32)
                nc.sync.dma_start(out=xs_t[:], in_=xs_d[i])
                g_t = gath_pool.tile([128, NIDX], F32)
                nc.gpsimd.ap_gather(g_t[:], xs_t[:], idx_sb[:, i, :],
                                    channels=128, num_elems=NE, d=1,
                                    num_idxs=NIDX)
                nc.vector.tensor_copy(cast_all[:, i, :], g_t[:])

            # -------- conv phase: 8 image pairs --------
            for bp in range(8):
                xps = []
                for b in (2 * bp, 2 * bp + 1):
                    xp = xpad_pool.tile([128, XPN], BF16, tag="xpad")
                    xpv = xp.rearrange("p (r c) -> p r c", c=XPW)
                    # zero the left/right pad columns of the lower half
                    nc.vector.memset(xpv[0:64, :, 0:1], 0.0)
                    nc.vector.memset(xpv[0:64, :, XPW - 1:XPW], 0.0)
                    # reshuffle: cast_all partition (16q+b), group i holds
                    # channel i*8+q -> xp partition q*8+i (c_at order).
                    for q in range(8):
                        srcq = cast_all[16 * q + b:16 * q + b + 1]  # [1,8,NIDX]
                        srcv = srcq.rearrange("p i (r c) -> p i r c", c=W)
                        dstv = xp[8 * q:8 * q + 8].rearrange(
                            "p (r c) -> p r c", c=XPW)
                        nc.sync.dma_start(out=dstv[:, :, 1:W + 1],
                                          in_=srcv[:])
                    # duplicate half = lower half shifted one row down
                    # (for the dy+1 taps), pad columns included.
                    nc.sync.dma_start(out=xp[64:128, 0:XPN - XPW],
                                      in_=xp[0:64, XPW:XPN])
                    xps.append(xpv)

                for h in range(2):
                    outt = out_pool.tile([128, 4 * 512], F32)
                    ps_tiles = [psum_pool.tile([128, 512], F32,
                                               name=f"ps{j}", tag=f"ps{j}")
                                for j in range(4)]
                    for ii, xpv in enumerate(xps):
                        cb = 64 * ii
                        # slots 0..2: fused (dy=0,dy=1) pairs, K=128
                        for dx in range(3):
                            for cch in range(4):
                                r0 = (h * 4 + cch) * 4
                                nc.tensor.matmul(
                                    ps_tiles[cch][cb:cb + 64, :],
                                    wt_sb[:, dx, :],
                                    xpv[:, r0:r0 + 4, dx:dx + W],
                                    start=(dx == 0), stop=False,
                                    tile_position=(0, cb))
                        # slots 3..5: dy=2 taps as plain K=64 matmuls
                        # (row-tiling via tile_position=(64,*) crashes the
                        # device on this stack)
                        for dx in range(3):
                            for cch in range(4):
                                r0 = (h * 4 + cch) * 4
                                nc.tensor.matmul(
                                    ps_tiles[cch][cb:cb + 64, :],
                                    wt_sb[0:64, 3 + dx, :],
                                    xpv[0:64, r0 + 2:r0 + 6, dx:dx + W],
                                    start=False, stop=(dx == 2),
                                    tile_position=(0, cb))
                    for cch in range(4):
                        nc.scalar.copy(outt[:, cch * 512:(cch + 1) * 512],
                                       ps_tiles[cch][:])
                    nc.sync.dma_start(
                        out=out_d[2 * bp:2 * bp + 2, :,
                                  h * 16:(h + 1) * 16, :],
                        in_=outt[:])

    nc.compile()
    return nc


def _prep_core_inputs(x, w, idx):
    """Host-side slicing/layout prep. Returns list of 8 in_maps."""
    bf16 = mybir.dt.np(BF16)
    x = np.ascontiguousarray(x, dtype=np.float32)
    idx = np.asarray(idx, dtype=np.int32)
    w = np.asarray(w, dtype=np.float32)

    # weights: lhsT[k, m] = w[m, c_at[k], dy, dx], c_at[p] = (p%8)*8 + p//8
    p = np.arange(64)
    c_at = (p % 8) * 8 + p // 8
    wp = w.transpose(1, 0, 2, 3)[c_at]      # [p, o, ky, kx]
    wts = np.zeros((128, 6, OUT), dtype=np.float32)
    for dx in range(3):
        wts[0:64, dx] = wp[:, :, 0, dx]
        wts[64:128, dx] = wp[:, :, 1, dx]
        wts[0:64, 3 + dx] = wp[:, :, 2, dx]
    wts = wts.astype(bf16)

    # x padded by 4 rows each side so every rq slice is a plain view
    xp = np.pad(x, ((0, 0), (0, 0), (4, 4), (0, 0)))
    idx_r = idx.reshape(C, H, W)

    in_maps = []
    idx16_by_rq = {}
    for k in range(N_CORES):
        bh, rq = k // 4, k % 4
        r0 = 32 * rq - 4                    # first source row (abs)
        sl = xp[bh * 16:(bh + 1) * 16, :, r0 + 4:r0 + 4 + SRC_ROWS, :]
        xs = np.ascontiguousarray(
            sl.reshape(BH_IMGS, 8, 8, NE).transpose(1, 2, 0, 3)
            .reshape(8, 128, NE))

        if rq not in idx16_by_rq:
            rows = np.arange(32 * rq - 1, 32 * rq + 33)
            valid = (rows >= 0) & (rows < H)
            flat = np.empty((C, G_ROWS, W), dtype=np.int32)
            flat[:, valid, :] = idx_r[:, rows[valid], :] - r0 * W
            inv = ~valid
            if inv.any():
                flat[:, inv, :] = ((rows[inv] - r0) * W)[None, :, None] + \
                    np.arange(W)[None, None, :]
            li = flat.reshape(8, 8, NIDX).astype(np.int16)
            idx16_by_rq[rq] = np.ascontiguousarray(
                li.reshape(8, 8, NIDX_W, 16).transpose(0, 1, 3, 2)
                .reshape(8, 128, NIDX_W))
        in_maps.append({"xs": xs, "idx16": idx16_by_rq[rq], "wts": wts})
    return in_maps


def get_compiled():
    if "nc" not in _CACHE:
        _CACHE["nc"] = _build_nc()
    return _CACHE["nc"]


def kernel(x, w, idx):
    nc = get_compiled()
    in_maps = _prep_core_inputs(x, w, idx)
    res = run_bass_kernel_spmd(nc, in_maps, list(range(N_CORES)))
    out = np.empty((B, OUT, H, W), dtype=np.float32)
    for k in range(N_CORES):
        bh, rq = k // 4, k % 4
        out[bh * 16:(bh + 1) * 16, :, 32 * rq:32 * rq + 32, :] = \
            res.results[k]["out"]
    return out


# revision 13
# speedup vs baseline: 28.5758x; 28.5758x over previous
"""Trainium2 Bass kernel for nn_Dconv_localshuffle.

Computes: per-channel spatial gather (block-local permutation within 4x4
blocks) followed by a 3x3 conv (stride 1, pad 1, no bias).

Sharding (8 NeuronCores, no cross-core comms):
  grid = (batch-half bh in {0,1}) x (row-quarter rq in {0..3})
  core k = bh*4 + rq handles images [bh*16, bh*16+16) and output rows
  [32*rq, 32*rq+32), all 64 channels. Halo rows are loaded redundantly
  (each core receives pre-sliced input), so cores are fully independent.

Per-core pipeline:
  DMA-in f32 tiles (partition = (ch-group, image)) ->
  GpSimd ap_gather (per-channel indices, shared across the 16 images on
  each Q7 core's partitions) -> DVE cast f32->bf16 ->
  stage to HBM in [img, ch, pos] layout (full-width DMAs; an SBUF->SBUF
  partition reshuffle would need 128 tiny single-partition transfers) ->
  per-image load into zero-padded conv layout + duplicate on partitions
  64-127 shifted one row down ->
  3x3 conv: 3 fused K=128 (dy=0,1) taps + 3 plain K=64 dy=2 taps, two
  images at a time via PE column tiling -> PSUM -> ACT evac -> DMA-out.
  DMAs are spread over the SP and ACT HWDGE rings + GpSimd SWDGE.
"""

import numpy as np

import concourse.bass as bass
import concourse.bacc as bacc
import concourse.mybir as mybir
from concourse import tile
from concourse.bass_utils import run_bass_kernel_spmd

# Problem constants (hardcoded per contract).
B, C, H, W = 32, 64, 128, 128
OUT = 64
N_CORES = 8
BH_IMGS = 16          # images per core
RQ_ROWS = 32          # output rows per core
SRC_ROWS = 40         # source rows loaded: [32*rq-4, 32*rq+36)
G_ROWS = 34           # gathered rows: [32*rq-1, 32*rq+33)
NE = SRC_ROWS * W     # 5120 gather source elems per partition
NIDX = G_ROWS * W     # 4352 gather indices per Q7 core
NIDX_W = NIDX // 16   # 272 wrapped idx columns
XPW = W + 2           # 130 padded row width
XPN = G_ROWS * XPW    # 4420 x_pad elems per partition

F32 = mybir.dt.float32
BF16 = mybir.dt.bfloat16
I16 = mybir.dt.int16

_CACHE = {}


def _build_nc():
    """Build + compile the (identical for all cores) Bass program."""
    nc = bacc.Bacc("TRN2", target_bir_lowering=False, debug=False,
                   num_devices=N_CORES)
    xs_d = nc.dram_tensor("xs", [8, 128, NE], F32, kind="ExternalInput")
    idx_d = nc.dram_tensor("idx16", [8, 128, NIDX_W], I16, kind="ExternalInput")
    wts_d = nc.dram_tensor("wts", [128, 6, OUT], BF16, kind="ExternalInput")
    out_d = nc.dram_tensor("out", [BH_IMGS, OUT, RQ_ROWS, W], F32,
                           kind="ExternalOutput")
    stage_d = nc.dram_tensor("stage", [BH_IMGS, C, NIDX], BF16)

    with tile.TileContext(nc) as tc:
        with (
            tc.tile_pool(name="const", bufs=1) as const_pool,
            tc.tile_pool(name="xs", bufs=3) as xs_pool,
            tc.tile_pool(name="gath", bufs=2) as gath_pool,
            tc.tile_pool(name="castb", bufs=2) as cast_pool,
            tc.tile_pool(name="xpad", bufs=4) as xpad_pool,
            tc.tile_pool(name="outsb", bufs=2) as out_pool,
            tc.tile_pool(name="psum", bufs=2, space=bass.MemorySpace.PSUM) as psum_pool,
        ):
            idx_sb = const_pool.tile([128, 8, NIDX_W], I16)
            for i in range(8):
                nc.sync.dma_start(out=idx_sb[:, i, :], in_=idx_d[i])
            wt_sb = const_pool.tile([128, 6, OUT], BF16)
            nc.sync.dma_start(out=wt_sb[:], in_=wts_d[:])

            # -------- gather phase: 8 channel-groups --------
            stage_cb = stage_d.rearrange("b c n -> c b n")
            for i in range(8):
                xs_t = xs_pool.tile([128, NE], F32)
                nc.sync.dma_start(out=xs_t[:], in_=xs_d[i])
                g_t = gath_pool.tile([128, NIDX], F32)
                nc.gpsimd.ap_gather(g_t[:], xs_t[:], idx_sb[:, i, :],
                                    channels=128, num_elems=NE, d=1,
                                    num_idxs=NIDX)
                c_t = cast_pool.tile([128, NIDX], BF16)
                nc.vector.tensor_copy(c_t[:], g_t[:])
                # stage[b, i*8+q, :] <- c_t[16q+b, :], full-width write
                nc.scalar.dma_start(out=stage_cb[i * 8:i * 8 + 8],
                                    in_=c_t[:])

            # -------- conv phase: 8 image pairs --------
            for bp in range(8):
                xps = []
                for b in (2 * bp, 2 * bp + 1):
                    xp = xpad_pool.tile([128, XPN], BF16, tag="xpad")
                    xpv = xp.rearrange("p (r c) -> p r c", c=XPW)
                    # zero the left/right pad columns of the lower half
                    nc.vector.memset(xpv[0:64, :, 0:1], 0.0)
                    nc.vector.memset(xpv[0:64, :, XPW - 1:XPW], 0.0)
                    eng = nc.sync if b % 2 == 0 else nc.scalar
                    srcb = stage_d[b].rearrange("c (r w) -> c r w", w=W)
                    eng.dma_start(out=xpv[0:64, :, 1:W + 1], in_=srcb[:])
                    # duplicate half = lower half shifted one row down
                    # (for the dy+1 taps), pad columns included. SWDGE ring.
                    nc.gpsimd.dma_start(out=xp[64:128, 0:XPN - XPW],
                                        in_=xp[0:64, XPW:XPN])
                    xps.append(xpv)

                for h in range(2):
                    outt = out_pool.tile([128, 4 * 512], F32)
                    ps_tiles = [psum_pool.tile([128, 512], F32,
                                               name=f"ps{j}", tag=f"ps{j}")
                                for j in range(4)]
                    for ii, xpv in enumerate(xps):
                        cb = 64 * ii
                        # slots 0..2: fused (dy=0,dy=1) pairs, K=128
                        for dx in range(3):
                            for cch in range(4):
                                r0 = (h * 4 + cch) * 4
                                nc.tensor.matmul(
                                    ps_tiles[cch][cb:cb + 64, :],
                                    wt_sb[:, dx, :],
                                    xpv[:, r0:r0 + 4, dx:dx + W],
                                    start=(dx == 0), stop=False,
                                    tile_position=(0, cb))
                        # slots 3..5: dy=2 taps as plain K=64 matmuls
                        # (row-tiling via tile_position=(64,*) crashes the
                        # device on this stack)
                        for dx in range(3):
                            for cch in range(4):
                                r0 = (h * 4 + cch) * 4
                                nc.tensor.matmul(
                                    ps_tiles[cch][cb:cb + 64, :],
                                    wt_sb[0:64, 3 + dx, :],
                                    xpv[0:64, r0 + 2:r0 + 6, dx:dx + W],
                                    start=False, stop=(dx == 2),
                                    tile_position=(0, cb))
                    for cch in range(4):
                        nc.scalar.copy(outt[:, cch * 512:(cch + 1) * 512],
                                       ps_tiles[cch][:])
                    eng = nc.scalar if bp % 2 == 0 else nc.sync
                    eng.dma_start(
                        out=out_d[2 * bp:2 * bp + 2, :,
                                  h * 16:(h + 1) * 16, :],
                        in_=outt[:])

    nc.compile()
    return nc


def _prep_core_inputs(x, w, idx):
    """Host-side slicing/layout prep. Returns list of 8 in_maps."""
    bf16 = mybir.dt.np(BF16)
    x = np.ascontiguousarray(x, dtype=np.float32)
    idx = np.asarray(idx, dtype=np.int32)
    w = np.asarray(w, dtype=np.float32)

    # weights: lhsT[k, m] = w[m, c_at[k], dy, dx], c_at[p] = (p%8)*8 + p//8
    p = np.arange(64)
    c_at = (p % 8) * 8 + p // 8
    wp = w.transpose(1, 0, 2, 3)[c_at]      # [p, o, ky, kx]
    wts = np.zeros((128, 6, OUT), dtype=np.float32)
    for dx in range(3):
        wts[0:64, dx] = wp[:, :, 0, dx]
        wts[64:128, dx] = wp[:, :, 1, dx]
        wts[0:64, 3 + dx] = wp[:, :, 2, dx]
    wts = wts.astype(bf16)

    # x padded by 4 rows each side so every rq slice is a plain view
    xp = np.pad(x, ((0, 0), (0, 0), (4, 4), (0, 0)))
    idx_r = idx.reshape(C, H, W)

    in_maps = []
    idx16_by_rq = {}
    for k in range(N_CORES):
        bh, rq = k // 4, k % 4
        r0 = 32 * rq - 4                    # first source row (abs)
        sl = xp[bh * 16:(bh + 1) * 16, :, r0 + 4:r0 + 4 + SRC_ROWS, :]
        xs = np.ascontiguousarray(
            sl.reshape(BH_IMGS, 8, 8, NE).transpose(1, 2, 0, 3)
            .reshape(8, 128, NE))

        if rq not in idx16_by_rq:
            rows = np.arange(32 * rq - 1, 32 * rq + 33)
            valid = (rows >= 0) & (rows < H)
            flat = np.empty((C, G_ROWS, W), dtype=np.int32)
            flat[:, valid, :] = idx_r[:, rows[valid], :] - r0 * W
            inv = ~valid
            if inv.any():
                flat[:, inv, :] = ((rows[inv] - r0) * W)[None, :, None] + \
                    np.arange(W)[None, None, :]
            li = flat.reshape(8, 8, NIDX).astype(np.int16)
            idx16_by_rq[rq] = np.ascontiguousarray(
                li.reshape(8, 8, NIDX_W, 16).transpose(0, 1, 3, 2)
                .reshape(8, 128, NIDX_W))
        in_maps.append({"xs": xs, "idx16": idx16_by_rq[rq], "wts": wts})
    return in_maps


def get_compiled():
    if "nc" not in _CACHE:
        _CACHE["nc"] = _build_nc()
    return _CACHE["nc"]


def kernel(x, w, idx):
    nc = get_compiled()
    in_maps = _prep_core_inputs(x, w, idx)
    res = run_bass_kernel_spmd(nc, in_maps, list(range(N_CORES)))
    out = np.empty((B, OUT, H, W), dtype=np.float32)
    for k in range(N_CORES):
        bh, rq = k // 4, k % 4
        out[bh * 16:(bh + 1) * 16, :, 32 * rq:32 * rq + 32, :] = \
            res.results[k]["out"]
    return out


# revision 14
# speedup vs baseline: 33.7804x; 1.1821x over previous
"""Trainium2 Bass kernel for nn_Dconv_localshuffle.

Computes: per-channel spatial gather (block-local permutation within 4x4
blocks) followed by a 3x3 conv (stride 1, pad 1, no bias).

Sharding (8 NeuronCores, no cross-core comms):
  grid = (batch-half bh in {0,1}) x (row-quarter rq in {0..3})
  core k = bh*4 + rq handles images [bh*16, bh*16+16) and output rows
  [32*rq, 32*rq+32), all 64 channels. Halo rows are loaded redundantly
  (each core receives pre-sliced input), so cores are fully independent.

Per-core pipeline:
  DMA-in f32 tiles (partition = (ch-group, image)) ->
  GpSimd ap_gather (per-channel indices, shared across the 16 images on
  each Q7 core's partitions) -> DVE cast f32->bf16 ->
  stage to HBM in [img, ch, pos] layout (full-width DMAs; an SBUF->SBUF
  partition reshuffle would need 128 tiny single-partition transfers) ->
  per-image load into zero-padded conv layout + duplicate on partitions
  64-127 shifted one row down ->
  3x3 conv: 3 fused K=128 (dy=0,1) taps + 3 plain K=64 dy=2 taps, two
  images at a time via PE column tiling -> PSUM -> ACT evac -> DMA-out.
  DMAs are spread over the SP and ACT HWDGE rings + GpSimd SWDGE.
"""

import numpy as np

import concourse.bass as bass
import concourse.bacc as bacc
import concourse.mybir as mybir
from concourse import tile
from concourse.bass_utils import run_bass_kernel_spmd

# Problem constants (hardcoded per contract).
B, C, H, W = 32, 64, 128, 128
OUT = 64
N_CORES = 8
BH_IMGS = 16          # images per core
RQ_ROWS = 32          # output rows per core
SRC_ROWS = 40         # source rows loaded: [32*rq-4, 32*rq+36)
G_ROWS = 34           # gathered rows: [32*rq-1, 32*rq+33)
NE = SRC_ROWS * W     # 5120 gather source elems per partition
NIDX = G_ROWS * W     # 4352 gather indices per Q7 core
NIDX_W = NIDX // 16   # 272 wrapped idx columns
XPW = W + 2           # 130 padded row width
XPN = G_ROWS * XPW    # 4420 x_pad elems per partition

F32 = mybir.dt.float32
BF16 = mybir.dt.bfloat16
I16 = mybir.dt.int16

_CACHE = {}


def _build_nc():
    """Build + compile the (identical for all cores) Bass program."""
    nc = bacc.Bacc("TRN2", target_bir_lowering=False, debug=False,
                   num_devices=N_CORES)
    xs_d = nc.dram_tensor("xs", [8, 128, NE], F32, kind="ExternalInput")
    idx_d = nc.dram_tensor("idx16", [8, 128, NIDX_W], I16, kind="ExternalInput")
    wts_d = nc.dram_tensor("wts", [128, 6, OUT], BF16, kind="ExternalInput")
    out_d = nc.dram_tensor("out", [BH_IMGS, OUT, RQ_ROWS, W], F32,
                           kind="ExternalOutput")
    stage_d = nc.dram_tensor("stage", [BH_IMGS, C, NIDX], BF16)

    with tile.TileContext(nc) as tc:
        with (
            tc.tile_pool(name="const", bufs=1) as const_pool,
            tc.tile_pool(name="xs", bufs=3) as xs_pool,
            tc.tile_pool(name="gath", bufs=2) as gath_pool,
            tc.tile_pool(name="castb", bufs=2) as cast_pool,
            tc.tile_pool(name="xpad", bufs=4) as xpad_pool,
            tc.tile_pool(name="outsb", bufs=2) as out_pool,
            tc.tile_pool(name="psum", bufs=2, space=bass.MemorySpace.PSUM) as psum_pool,
        ):
            idx_sb = const_pool.tile([128, 8, NIDX_W], I16)
            for i in range(8):
                nc.sync.dma_start(out=idx_sb[:, i, :], in_=idx_d[i])
            wt_sb = const_pool.tile([128, 6, OUT], BF16)
            nc.sync.dma_start(out=wt_sb[:], in_=wts_d[:])

            # -------- gather phase: 8 channel-groups --------
            stage_cb = stage_d.rearrange("b c n -> c b n")
            for i in range(8):
                xs_t = xs_pool.tile([128, NE], F32)
                nc.sync.dma_start(out=xs_t[:], in_=xs_d[i])
                g_t = gath_pool.tile([128, NIDX], F32)
                nc.gpsimd.ap_gather(g_t[:], xs_t[:], idx_sb[:, i, :],
                                    channels=128, num_elems=NE, d=1,
                                    num_idxs=NIDX)
                c_t = cast_pool.tile([128, NIDX], BF16)
                nc.vector.tensor_copy(c_t[:], g_t[:])
                # stage[b, i*8+q, :] <- c_t[16q+b, :], full-width write
                nc.scalar.dma_start(out=stage_cb[i * 8:i * 8 + 8],
                                    in_=c_t[:])

            # -------- conv phase: 8 image pairs --------
            for bp in range(8):
                xps = []
                for b in (2 * bp, 2 * bp + 1):
                    xp = xpad_pool.tile([128, XPN], BF16, tag="xpad")
                    xpv = xp.rearrange("p (r c) -> p r c", c=XPW)
                    # zero the left/right pad columns of the lower half
                    nc.vector.memset(xpv[0:64, :, 0:1], 0.0)
                    nc.vector.memset(xpv[0:64, :, XPW - 1:XPW], 0.0)
                    eng = nc.sync if b % 2 == 0 else nc.scalar
                    srcb = stage_d[b].rearrange("c (r w) -> c r w", w=W)
                    eng.dma_start(out=xpv[0:64, :, 1:W + 1], in_=srcb[:])
                    # duplicate half = lower half shifted one row down
                    # (for the dy+1 taps), pad columns included. SWDGE ring.
                    nc.gpsimd.dma_start(out=xp[64:128, 0:XPN - XPW],
                                        in_=xp[0:64, XPW:XPN])
                    xps.append(xpv)

                for h in range(2):
                    outt = out_pool.tile([128, 4 * 512], F32)
                    ps_tiles = [psum_pool.tile([128, 512], F32,
                                               name=f"ps{j}", tag=f"ps{j}")
                                for j in range(4)]
                    for ii, xpv in enumerate(xps):
                        cb = 64 * ii
                        # slots 0..2: fused (dy=0,dy=1) pairs, K=128
                        for dx in range(3):
                            for cch in range(4):
                                r0 = (h * 4 + cch) * 4
                                nc.tensor.matmul(
                                    ps_tiles[cch][cb:cb + 64, :],
                                    wt_sb[:, dx, :],
                                    xpv[:, r0:r0 + 4, dx:dx + W],
                                    start=(dx == 0), stop=False,
                                    tile_position=(0, cb))
                        # slots 3..5: dy=2 taps as plain K=64 matmuls
                        # (row-tiling via tile_position=(64,*) crashes the
                        # device on this stack)
                        for dx in range(3):
                            for cch in range(4):
                                r0 = (h * 4 + cch) * 4
                                nc.tensor.matmul(
                                    ps_tiles[cch][cb:cb + 64, :],
                                    wt_sb[0:64, 3 + dx, :],
                                    xpv[0:64, r0 + 2:r0 + 6, dx:dx + W],
                                    start=False, stop=(dx == 2),
                                    tile_position=(0, cb))
                    for cch in range(4):
                        nc.scalar.copy(outt[:, cch * 512:(cch + 1) * 512],
                                       ps_tiles[cch][:])
                    eng = nc.scalar if bp % 2 == 0 else nc.sync
                    eng.dma_start(
                        out=out_d[2 * bp:2 * bp + 2, :,
                                  h * 16:(h + 1) * 16, :],
                        in_=outt[:])

    nc.compile()
    return nc


def _prep_core_inputs(x, w, idx):
    """Host-side slicing/layout prep. Returns list of 8 in_maps."""
    bf16 = mybir.dt.np(BF16)
    x = np.ascontiguousarray(x, dtype=np.float32)
    idx = np.asarray(idx, dtype=np.int32)
    w = np.asarray(w, dtype=np.float32)

    # weights: lhsT[k, m] = w[m, k, dy, dx] — the HBM staging step stores
    # channels in natural order, so conv partition p holds channel p.
    wp = w.transpose(1, 0, 2, 3)            # [c, o, ky, kx]
    wts = np.zeros((128, 6, OUT), dtype=np.float32)
    for dx in range(3):
        wts[0:64, dx] = wp[:, :, 0, dx]
        wts[64:128, dx] = wp[:, :, 1, dx]
        wts[0:64, 3 + dx] = wp[:, :, 2, dx]
    wts = wts.astype(bf16)

    # x padded by 4 rows each side so every rq slice is a plain view
    xp = np.pad(x, ((0, 0), (0, 0), (4, 4), (0, 0)))
    idx_r = idx.reshape(C, H, W)

    in_maps = []
    idx16_by_rq = {}
    for k in range(N_CORES):
        bh, rq = k // 4, k % 4
        r0 = 32 * rq - 4                    # first source row (abs)
        sl = xp[bh * 16:(bh + 1) * 16, :, r0 + 4:r0 + 4 + SRC_ROWS, :]
        xs = np.ascontiguousarray(
            sl.reshape(BH_IMGS, 8, 8, NE).transpose(1, 2, 0, 3)
            .reshape(8, 128, NE))

        if rq not in idx16_by_rq:
            rows = np.arange(32 * rq - 1, 32 * rq + 33)
            valid = (rows >= 0) & (rows < H)
            flat = np.empty((C, G_ROWS, W), dtype=np.int32)
            flat[:, valid, :] = idx_r[:, rows[valid], :] - r0 * W
            inv = ~valid
            if inv.any():
                flat[:, inv, :] = ((rows[inv] - r0) * W)[None, :, None] + \
                    np.arange(W)[None, None, :]
            li = flat.reshape(8, 8, NIDX).astype(np.int16)
            idx16_by_rq[rq] = np.ascontiguousarray(
                li.reshape(8, 8, NIDX_W, 16).transpose(0, 1, 3, 2)
                .reshape(8, 128, NIDX_W))
        in_maps.append({"xs": xs, "idx16": idx16_by_rq[rq], "wts": wts})
    return in_maps


def get_compiled():
    if "nc" not in _CACHE:
        _CACHE["nc"] = _build_nc()
    return _CACHE["nc"]


def kernel(x, w, idx):
    nc = get_compiled()
    in_maps = _prep_core_inputs(x, w, idx)
    res = run_bass_kernel_spmd(nc, in_maps, list(range(N_CORES)))
    out = np.empty((B, OUT, H, W), dtype=np.float32)
    for k in range(N_CORES):
        bh, rq = k // 4, k % 4
        out[bh * 16:(bh + 1) * 16, :, 32 * rq:32 * rq + 32, :] = \
            res.results[k]["out"]
    return out
